# revision 18
# baseline (speedup 1.0000x reference)
"""MoE layer kernel for Trainium2, 8 NeuronCores, data-parallel over tokens.

Problem: x (4, 4096, 1024), router Wr (1024, 8) + br, experts W1 (8,1024,1024)
+ b1, W2 (8,1024,1024) + b2, top-2 softmax routing, dense-equivalent output
out (4, 4096, 1024).

Sharding: 16384 tokens split 8 ways (2048 tokens/core); expert weights
replicated. Math per core (dense over experts, exact vs reference):
  logits = x @ Wr + br ; top2 ; c0 = sigmoid(m1-m2), c1 = 1-c0
  coef_full[t,e] = c0*[e==argmax1] + c1*[e==argmax2]
  out = sum_e coef_full[:,e] * (relu(x @ W1[e] + b1[e]) @ W2[e] + b2[e])
     = sum_e coef_full[:,e] * (relu(...) @ W2[e])  +  coef_full @ b2
Matmuls run in float32r (full PE rate, ~fp22 mantissa).
"""
import sys

sys.path.insert(0, "/opt/trn_rl_repo")

import numpy as np
import concourse.bass as bass
import concourse.mybir as mybir
import concourse.tile as tile
from concourse import bacc
from concourse.bass_utils import run_bass_kernel_spmd
from concourse.masks import make_identity

dt = mybir.dt
AF = mybir.ActivationFunctionType
ALU = mybir.AluOpType

NCORES = 8
B, NOBJ, D = 4, 4096, 1024
H = O = 1024
E = 8
TOK = B * NOBJ          # 16384 tokens total
T = TOK // NCORES       # 2048 tokens per core
TH = T // 2             # half = 1024 tokens (SBUF fits a half)
P = 128

_NC_CACHE = {}


def build_nc():
    if "nc" in _NC_CACHE:
        return _NC_CACHE["nc"]
    nc = bacc.Bacc("TRN2", target_bir_lowering=False, debug=False)

    xT = nc.dram_tensor("xT", [D, T], dt.float32r, kind="ExternalInput")
    xThi = nc.dram_tensor("xThi", [D, T], dt.float32r, kind="ExternalInput")
    xTlo = nc.dram_tensor("xTlo", [D, T], dt.float32r, kind="ExternalInput")
    wrhi = nc.dram_tensor("wrhi", [D, E], dt.float32r, kind="ExternalInput")
    wrlo = nc.dram_tensor("wrlo", [D, E], dt.float32r, kind="ExternalInput")
    brc = nc.dram_tensor("brc", [E, 1], dt.float32, kind="ExternalInput")
    w1 = nc.dram_tensor("w1", [E, D, H], dt.float32r, kind="ExternalInput")
    b1c = nc.dram_tensor("b1c", [P, E * (H // P)], dt.float32, kind="ExternalInput")
    w2 = nc.dram_tensor("w2", [E, H, O], dt.float32r, kind="ExternalInput")
    b2 = nc.dram_tensor("b2", [E, O], dt.float32r, kind="ExternalInput")
    out = nc.dram_tensor("out", [T, O], dt.float32, kind="ExternalOutput")

    ND = D // P   # 8 d-slices
    NH = H // P   # 8 h-slices
    NT = TH // P  # 8 token tiles per half
    NC2 = TH // 512  # 2 token chunks of 512 per half
    NOC = O // 512   # 2 o chunks

    with tile.TileContext(nc) as tc:
        with (
            tc.tile_pool(name="const", bufs=1) as cpool,
            tc.tile_pool(name="xt", bufs=ND + 1) as xt_pool,
            tc.tile_pool(name="w1p", bufs=6) as w1_pool,
            tc.tile_pool(name="w2p", bufs=NH + 2) as w2_pool,
            tc.tile_pool(name="hp", bufs=NH + 1) as h_pool,
            tc.tile_pool(name="acc", bufs=NT) as acc_pool,
            tc.tile_pool(name="rt", bufs=2) as rt_pool,
            tc.tile_pool(name="cfp", bufs=NT + 1) as cf_pool,
            tc.tile_pool(name="ps1", bufs=4, space="PSUM") as ps1,
            tc.tile_pool(name="ps2", bufs=2, space="PSUM") as ps2,
            tc.tile_pool(name="psm", bufs=1, space="PSUM") as psm,
        ):
            ident = cpool.tile([P, P], dt.float32)
            make_identity(nc, ident[:])
            # hi/lo split of router inputs is done on host: hi parts are
            # m11-exact so the PE's fp32r read rounding is a no-op and the
            # 4 accumulated hi/lo products give ~fp32-exact logits
            wr_hi = cpool.tile([P, ND * E], dt.float32r)
            wr_lo = cpool.tile([P, ND * E], dt.float32r)
            for ds in range(ND):
                nc.sync.dma_start(wr_hi[:, ds * E:(ds + 1) * E], wrhi[ds * P:(ds + 1) * P, :])
                nc.sync.dma_start(wr_lo[:, ds * E:(ds + 1) * E], wrlo[ds * P:(ds + 1) * P, :])
            brc_sb = cpool.tile([E, 1], dt.float32)
            nc.sync.dma_start(brc_sb[:], brc[:])
            b1c_sb = cpool.tile([P, E * NH], dt.float32)
            nc.sync.dma_start(b1c_sb[:], b1c[:])
            b2_sb = cpool.tile([E, O], dt.float32r)
            nc.sync.dma_start(b2_sb[:], b2[:])

            for half in range(2):
                t0 = half * TH
                # ---- X^T tiles for this half: 8 x (128, 1024), float32r
                xt = []
                for ds in range(ND):
                    xti = xt_pool.tile([P, TH], dt.float32r, tag="xt")
                    nc.gpsimd.dma_start(xti[:], xT[ds * P:(ds + 1) * P, t0:t0 + TH])
                    xt.append(xti)

                # ---- router: logitsT (8, TH) then transpose to token-major
                logitsT = rt_pool.tile([E, TH], dt.float32, tag="logitsT")
                for c in range(NC2):
                    cs = slice(c * 512, (c + 1) * 512)
                    pr = psm.tile([E, 512], dt.float32, tag="psr")
                    for ds in range(ND):
                        xhi = rt_pool.tile([P, 512], dt.float32r, tag="xhi")
                        xlo = rt_pool.tile([P, 512], dt.float32r, tag="xlo")
                        nc.sync.dma_start(xhi[:], xThi[ds * P:(ds + 1) * P, t0 + c * 512:t0 + (c + 1) * 512])
                        nc.sync.dma_start(xlo[:], xTlo[ds * P:(ds + 1) * P, t0 + c * 512:t0 + (c + 1) * 512])
                        whi_s = wr_hi[:, ds * E:(ds + 1) * E]
                        wlo_s = wr_lo[:, ds * E:(ds + 1) * E]
                        for mi, (wop, xop) in enumerate(
                                [(whi_s, xhi), (wlo_s, xhi), (whi_s, xlo), (wlo_s, xlo)]):
                            nc.tensor.matmul(
                                out=pr[:], lhsT=wop, rhs=xop[:],
                                start=(ds == 0 and mi == 0),
                                stop=(ds == ND - 1 and mi == 3),
                            )
                    nc.vector.tensor_scalar(logitsT[:, cs], pr[:], brc_sb[:, 0:1], None, op0=ALU.add)

                coef = []    # token-major coef_full tiles (128, 8) fp32
                coefT = rt_pool.tile([E, TH], dt.float32r, tag="coefT")
                for tt in range(NT):
                    ts_ = slice(tt * P, (tt + 1) * P)
                    pl = psm.tile([P, E], dt.float32, tag="pst")
                    nc.tensor.transpose(out=pl[:], in_=logitsT[:, ts_], identity=ident[:E, :E])
                    lg = rt_pool.tile([P, E], dt.float32, tag="lg")
                    nc.scalar.copy(lg[:], pl[:])
                    top = rt_pool.tile([P, 8], dt.float32, tag="top")
                    topi = rt_pool.tile([P, 8], dt.uint32, tag="topi")
                    nc.vector.max_with_indices(top[:], topi[:], lg[:])
                    m1, m2 = top[:, 0:1], top[:, 1:2]
                    d01 = rt_pool.tile([P, 1], dt.float32, tag="d01")
                    nc.vector.tensor_sub(d01[:], m1, m2)
                    c0 = rt_pool.tile([P, 1], dt.float32, tag="c0")
                    nc.scalar.activation(out=c0[:], in_=d01[:], func=AF.Sigmoid)
                    c1 = rt_pool.tile([P, 1], dt.float32, tag="c1")
                    nc.vector.tensor_scalar(c1[:], c0[:], -1.0, 1.0, op0=ALU.mult, op1=ALU.add)
                    eq0 = rt_pool.tile([P, E], dt.float32, tag="eq0")
                    nc.vector.tensor_scalar(eq0[:], lg[:], m1, None, op0=ALU.is_equal)
                    eq1 = rt_pool.tile([P, E], dt.float32, tag="eq1")
                    nc.vector.tensor_scalar(eq1[:], lg[:], m2, None, op0=ALU.is_equal)
                    cf = cf_pool.tile([P, E], dt.float32, tag="cf")
                    nc.vector.tensor_scalar(cf[:], eq0[:], c0[:], None, op0=ALU.mult)
                    nc.vector.scalar_tensor_tensor(
                        out=cf[:], in0=eq1[:], scalar=c1[:], in1=cf[:],
                        op0=ALU.mult, op1=ALU.add,
                    )
                    coef.append(cf)
                    # transpose coef tile -> coefT columns (cast to f32r via copy)
                    pc = psm.tile([E, P], dt.float32, tag="pst")
                    nc.tensor.transpose(out=pc[:], in_=cf[:], identity=ident[:])
                    nc.vector.tensor_copy(coefT[:, ts_], pc[:])

                # ---- init outacc with coef_full @ b2  (K=8 matmul)
                outacc = []
                for tt in range(NT):
                    ts_ = slice(tt * P, (tt + 1) * P)
                    oa = acc_pool.tile([P, O], dt.float32, tag="acc")
                    for oc in range(NOC):
                        os_ = slice(oc * 512, (oc + 1) * 512)
                        pb = ps2.tile([P, 512], dt.float32, tag="ps2")
                        nc.tensor.matmul(out=pb[:], lhsT=coefT[:, ts_], rhs=b2_sb[:, os_],
                                         start=True, stop=True)
                        nc.scalar.copy(oa[:, os_], pb[:])
                    outacc.append(oa)

                # ---- experts
                for e in range(E):
                    # mm1: H^T = relu(W1[e]^T x^T + b1)  in h-groups of 4 slices
                    hbuf = []
                    for c in range(NC2):
                        cs = slice(c * 512, (c + 1) * 512)
                        for grp in range(2):
                            hs0 = grp * 4
                            pgrp = [ps1.tile([P, 512], dt.float32, tag="ps1", name=f"ps1_{hi}")
                                    for hi in range(4)]
                            for ds in range(ND):
                                w1t = w1_pool.tile([P, 512], dt.float32r, tag="w1")
                                nc.sync.dma_start(
                                    w1t[:], w1[e, ds * P:(ds + 1) * P, hs0 * P:(hs0 + 4) * P])
                                for hi in range(4):
                                    nc.tensor.matmul(
                                        out=pgrp[hi][:],
                                        lhsT=w1t[:, hi * P:(hi + 1) * P],
                                        rhs=xt[ds][:, cs],
                                        start=(ds == 0), stop=(ds == ND - 1),
                                    )
                            for hi in range(4):
                                hs = hs0 + hi
                                if c == 0:
                                    ht = h_pool.tile([P, TH], dt.float32r, tag="h")
                                    hbuf.append(ht)
                                nc.scalar.activation(
                                    out=hbuf[hs][:, cs], in_=pgrp[hi][:], func=AF.Relu,
                                    bias=b1c_sb[:, e * NH + hs:e * NH + hs + 1],
                                )
                    # reorder hbuf: created in order hs = 0,1,2,3 (c=0 grp0), 4..7
                    # mm2: out += coef_e * (H^T)^T W2[e]
                    for oc in range(NOC):
                        os_ = slice(oc * 512, (oc + 1) * 512)
                        w2ts = []
                        for hs in range(NH):
                            w2t = w2_pool.tile([P, 512], dt.float32r, tag="w2")
                            nc.sync.dma_start(
                                w2t[:], w2[e, hs * P:(hs + 1) * P, os_])
                            w2ts.append(w2t)
                        for tt in range(NT):
                            ts_ = slice(tt * P, (tt + 1) * P)
                            py = ps2.tile([P, 512], dt.float32, tag="ps2")
                            for hs in range(NH):
                                nc.tensor.matmul(
                                    out=py[:], lhsT=hbuf[hs][:, ts_], rhs=w2ts[hs][:],
                                    start=(hs == 0), stop=(hs == NH - 1),
                                )
                            nc.vector.scalar_tensor_tensor(
                                out=outacc[tt][:, os_], in0=py[:],
                                scalar=coef[tt][:, e:e + 1], in1=outacc[tt][:, os_],
                                op0=ALU.mult, op1=ALU.add,
                            )

                for tt in range(NT):
                    nc.sync.dma_start(out[t0 + tt * P:t0 + (tt + 1) * P, :], outacc[tt][:])

    nc.compile()
    _NC_CACHE["nc"] = nc
    return nc


def kernel(x, Wr, br, W1, b1, W2, b2):
    x = np.ascontiguousarray(np.asarray(x, dtype=np.float32))
    Wr = np.ascontiguousarray(np.asarray(Wr, dtype=np.float32))
    br = np.asarray(br, dtype=np.float32)
    W1 = np.ascontiguousarray(np.asarray(W1, dtype=np.float32))
    b1 = np.asarray(b1, dtype=np.float32)
    W2 = np.ascontiguousarray(np.asarray(W2, dtype=np.float32))
    b2 = np.ascontiguousarray(np.asarray(b2, dtype=np.float32))

    xf = x.reshape(TOK, D)
    b1c = np.ascontiguousarray(b1.reshape(E, H // P, P).transpose(2, 0, 1).reshape(P, E * (H // P)))
    brc = np.ascontiguousarray(br.reshape(E, 1))

    MASK11 = np.uint32(0xFFFFF000)
    xhi = (xf.view(np.uint32) & MASK11).view(np.float32)
    xlo = xf - xhi
    wrhi = (Wr.view(np.uint32) & MASK11).view(np.float32)
    wrlo = Wr - wrhi

    nc = build_nc()
    in_maps = []
    for c in range(NCORES):
        sl = slice(c * T, (c + 1) * T)
        in_maps.append({
            "xT": np.ascontiguousarray(xf[sl].T),
            "xThi": np.ascontiguousarray(xhi[sl].T),
            "xTlo": np.ascontiguousarray(xlo[sl].T),
            "wrhi": wrhi, "wrlo": wrlo,
            "brc": brc, "w1": W1, "b1c": b1c, "w2": W2, "b2": b2,
        })
    res = run_bass_kernel_spmd(nc, in_maps, core_ids=list(range(NCORES)))
    out = np.concatenate([res.results[c]["out"] for c in range(NCORES)], axis=0)
    return out.reshape(B, NOBJ, O)


# revision 22
# speedup vs baseline: 20.1449x; 20.1449x over previous
"""MoE layer kernel for Trainium2, 8 NeuronCores, data-parallel over tokens.

Problem: x (4, 4096, 1024), router Wr (1024, 8) + br, experts W1 (8,1024,1024)
+ b1, W2 (8,1024,1024) + b2, top-2 softmax routing, dense-equivalent output
out (4, 4096, 1024).

Sharding: 16384 tokens split 8 ways (2048 tokens/core); expert weights
replicated. Math per core (dense over experts, exact vs reference):
  logits = x @ Wr + br ; top2 ; c0 = sigmoid(m1-m2), c1 = 1-c0
  coef_full[t,e] = c0*[e==argmax1] + c1*[e==argmax2]
  out = sum_e coef_full[:,e] * (relu(x @ W1[e] + b1[e]) @ W2[e] + b2[e])
     = sum_e coef_full[:,e] * (relu(...) @ W2[e])  +  coef_full @ b2
Matmuls run in float32r (full PE rate, ~fp22 mantissa).
"""
import sys

sys.path.insert(0, "/opt/trn_rl_repo")

import numpy as np
import concourse.bass as bass
import concourse.mybir as mybir
import concourse.tile as tile
from concourse import bacc
from concourse.bass_utils import run_bass_kernel_spmd
from concourse.masks import make_identity

dt = mybir.dt
AF = mybir.ActivationFunctionType
ALU = mybir.AluOpType

NCORES = 8
B, NOBJ, D = 4, 4096, 1024
H = O = 1024
E = 8
TOK = B * NOBJ          # 16384 tokens total
T = TOK // NCORES       # 2048 tokens per core
TH = T // 2             # half = 1024 tokens (SBUF fits a half)
P = 128

_NC_CACHE = {}


def build_nc(body_reps=1):
    key = ("nc", body_reps)
    if key in _NC_CACHE:
        return _NC_CACHE[key]
    nc = bacc.Bacc("TRN2", target_bir_lowering=False, debug=False)

    xT = nc.dram_tensor("xT", [D, T], dt.float32r, kind="ExternalInput")
    xThi = nc.dram_tensor("xThi", [D, T], dt.float32r, kind="ExternalInput")
    xTlo = nc.dram_tensor("xTlo", [D, T], dt.float32r, kind="ExternalInput")
    wrhi = nc.dram_tensor("wrhi", [D, E], dt.float32r, kind="ExternalInput")
    wrlo = nc.dram_tensor("wrlo", [D, E], dt.float32r, kind="ExternalInput")
    brc = nc.dram_tensor("brc", [E, 1], dt.float32, kind="ExternalInput")
    w1 = nc.dram_tensor("w1", [E, D, H], dt.float32r, kind="ExternalInput")
    b1c = nc.dram_tensor("b1c", [P, E * (H // P)], dt.float32, kind="ExternalInput")
    w2 = nc.dram_tensor("w2", [E, H, O], dt.float32r, kind="ExternalInput")
    b2 = nc.dram_tensor("b2", [E, O], dt.float32r, kind="ExternalInput")
    out = nc.dram_tensor("out", [T, O], dt.float32, kind="ExternalOutput")

    ND = D // P   # 8 d-slices
    NH = H // P   # 8 h-slices
    NT = TH // P  # 8 token tiles per half
    NC2 = TH // 512  # 2 token chunks of 512 per half
    NOC = O // 512   # 2 o chunks

    with tile.TileContext(nc) as tc:
        with (
            tc.tile_pool(name="const", bufs=1) as cpool,
            tc.tile_pool(name="xt", bufs=ND + 1) as xt_pool,
            tc.tile_pool(name="w1p", bufs=6) as w1_pool,
            tc.tile_pool(name="w2p", bufs=NH + 2) as w2_pool,
            tc.tile_pool(name="hp", bufs=NH + 1) as h_pool,
            tc.tile_pool(name="acc", bufs=NT) as acc_pool,
            tc.tile_pool(name="rt", bufs=2) as rt_pool,
            tc.tile_pool(name="cfp", bufs=NT + 1) as cf_pool,
            tc.tile_pool(name="ps1", bufs=4, space="PSUM") as ps1,
            tc.tile_pool(name="ps2", bufs=2, space="PSUM") as ps2,
            tc.tile_pool(name="psm", bufs=1, space="PSUM") as psm,
        ):
            ident = cpool.tile([P, P], dt.float32)
            make_identity(nc, ident[:])
            # hi/lo split of router inputs is done on host: hi parts are
            # m11-exact so the PE's fp32r read rounding is a no-op and the
            # 4 accumulated hi/lo products give ~fp32-exact logits
            wr_hi = cpool.tile([P, ND * E], dt.float32r)
            wr_lo = cpool.tile([P, ND * E], dt.float32r)
            for ds in range(ND):
                nc.sync.dma_start(wr_hi[:, ds * E:(ds + 1) * E], wrhi[ds * P:(ds + 1) * P, :])
                nc.sync.dma_start(wr_lo[:, ds * E:(ds + 1) * E], wrlo[ds * P:(ds + 1) * P, :])
            brc_sb = cpool.tile([E, 1], dt.float32)
            nc.sync.dma_start(brc_sb[:], brc[:])
            b1c_sb = cpool.tile([P, E * NH], dt.float32)
            nc.sync.dma_start(b1c_sb[:], b1c[:])
            b2_sb = cpool.tile([E, O], dt.float32r)
            nc.sync.dma_start(b2_sb[:], b2[:])

            for rep in range(body_reps):
              for half in range(2):
                t0 = half * TH
                # ---- X^T tiles for this half: 8 x (128, 1024), float32r
                xt = []
                for ds in range(ND):
                    xti = xt_pool.tile([P, TH], dt.float32r, tag="xt")
                    nc.gpsimd.dma_start(xti[:], xT[ds * P:(ds + 1) * P, t0:t0 + TH])
                    xt.append(xti)

                # ---- router: logitsT (8, TH) then transpose to token-major
                logitsT = rt_pool.tile([E, TH], dt.float32, tag="logitsT")
                for c in range(NC2):
                    cs = slice(c * 512, (c + 1) * 512)
                    pr = psm.tile([E, 512], dt.float32, tag="psr")
                    for ds in range(ND):
                        xhi = rt_pool.tile([P, 512], dt.float32r, tag="xhi")
                        xlo = rt_pool.tile([P, 512], dt.float32r, tag="xlo")
                        nc.sync.dma_start(xhi[:], xThi[ds * P:(ds + 1) * P, t0 + c * 512:t0 + (c + 1) * 512])
                        nc.sync.dma_start(xlo[:], xTlo[ds * P:(ds + 1) * P, t0 + c * 512:t0 + (c + 1) * 512])
                        whi_s = wr_hi[:, ds * E:(ds + 1) * E]
                        wlo_s = wr_lo[:, ds * E:(ds + 1) * E]
                        for mi, (wop, xop) in enumerate(
                                [(whi_s, xhi), (wlo_s, xhi), (whi_s, xlo), (wlo_s, xlo)]):
                            nc.tensor.matmul(
                                out=pr[:], lhsT=wop, rhs=xop[:],
                                start=(ds == 0 and mi == 0),
                                stop=(ds == ND - 1 and mi == 3),
                            )
                    nc.vector.tensor_scalar(logitsT[:, cs], pr[:], brc_sb[:, 0:1], None, op0=ALU.add)

                coef = []    # token-major coef_full tiles (128, 8) fp32
                coefT = rt_pool.tile([E, TH], dt.float32r, tag="coefT")
                for tt in range(NT):
                    ts_ = slice(tt * P, (tt + 1) * P)
                    pl = psm.tile([P, E], dt.float32, tag="pst")
                    nc.tensor.transpose(out=pl[:], in_=logitsT[:, ts_], identity=ident[:E, :E])
                    lg = rt_pool.tile([P, E], dt.float32, tag="lg")
                    nc.scalar.copy(lg[:], pl[:])
                    top = rt_pool.tile([P, 8], dt.float32, tag="top")
                    topi = rt_pool.tile([P, 8], dt.uint32, tag="topi")
                    nc.vector.max_with_indices(top[:], topi[:], lg[:])
                    m1, m2 = top[:, 0:1], top[:, 1:2]
                    d01 = rt_pool.tile([P, 1], dt.float32, tag="d01")
                    nc.vector.tensor_sub(d01[:], m1, m2)
                    c0 = rt_pool.tile([P, 1], dt.float32, tag="c0")
                    nc.scalar.activation(out=c0[:], in_=d01[:], func=AF.Sigmoid)
                    c1 = rt_pool.tile([P, 1], dt.float32, tag="c1")
                    nc.vector.tensor_scalar(c1[:], c0[:], -1.0, 1.0, op0=ALU.mult, op1=ALU.add)
                    eq0 = rt_pool.tile([P, E], dt.float32, tag="eq0")
                    nc.vector.tensor_scalar(eq0[:], lg[:], m1, None, op0=ALU.is_equal)
                    eq1 = rt_pool.tile([P, E], dt.float32, tag="eq1")
                    nc.vector.tensor_scalar(eq1[:], lg[:], m2, None, op0=ALU.is_equal)
                    cf = cf_pool.tile([P, E], dt.float32, tag="cf")
                    nc.vector.tensor_scalar(cf[:], eq0[:], c0[:], None, op0=ALU.mult)
                    nc.vector.scalar_tensor_tensor(
                        out=cf[:], in0=eq1[:], scalar=c1[:], in1=cf[:],
                        op0=ALU.mult, op1=ALU.add,
                    )
                    coef.append(cf)
                    # transpose coef tile -> coefT columns (cast to f32r via copy)
                    pc = psm.tile([E, P], dt.float32, tag="pst")
                    nc.tensor.transpose(out=pc[:], in_=cf[:], identity=ident[:])
                    nc.vector.tensor_copy(coefT[:, ts_], pc[:])

                # ---- init outacc with coef_full @ b2  (K=8 matmul)
                outacc = []
                for tt in range(NT):
                    ts_ = slice(tt * P, (tt + 1) * P)
                    oa = acc_pool.tile([P, O], dt.float32, tag="acc")
                    for oc in range(NOC):
                        os_ = slice(oc * 512, (oc + 1) * 512)
                        pb = ps2.tile([P, 512], dt.float32, tag="ps2")
                        nc.tensor.matmul(out=pb[:], lhsT=coefT[:, ts_], rhs=b2_sb[:, os_],
                                         start=True, stop=True)
                        nc.scalar.copy(oa[:, os_], pb[:])
                    outacc.append(oa)

                # ---- experts
                for e in range(E):
                    # mm1: H^T = relu(W1[e]^T x^T + b1)  in h-groups of 4 slices
                    hbuf = []
                    for c in range(NC2):
                        cs = slice(c * 512, (c + 1) * 512)
                        for grp in range(2):
                            hs0 = grp * 4
                            pgrp = [ps1.tile([P, 512], dt.float32, tag="ps1", name=f"ps1_{hi}")
                                    for hi in range(4)]
                            for ds in range(ND):
                                w1t = w1_pool.tile([P, 512], dt.float32r, tag="w1")
                                nc.sync.dma_start(
                                    w1t[:], w1[e, ds * P:(ds + 1) * P, hs0 * P:(hs0 + 4) * P])
                                for hi in range(4):
                                    nc.tensor.matmul(
                                        out=pgrp[hi][:],
                                        lhsT=w1t[:, hi * P:(hi + 1) * P],
                                        rhs=xt[ds][:, cs],
                                        start=(ds == 0), stop=(ds == ND - 1),
                                    )
                            for hi in range(4):
                                hs = hs0 + hi
                                if c == 0:
                                    ht = h_pool.tile([P, TH], dt.float32r, tag="h")
                                    hbuf.append(ht)
                                nc.scalar.activation(
                                    out=hbuf[hs][:, cs], in_=pgrp[hi][:], func=AF.Relu,
                                    bias=b1c_sb[:, e * NH + hs:e * NH + hs + 1],
                                )
                    # reorder hbuf: created in order hs = 0,1,2,3 (c=0 grp0), 4..7
                    # mm2: out += coef_e * (H^T)^T W2[e]
                    for oc in range(NOC):
                        os_ = slice(oc * 512, (oc + 1) * 512)
                        w2ts = []
                        for hs in range(NH):
                            w2t = w2_pool.tile([P, 512], dt.float32r, tag="w2")
                            nc.sync.dma_start(
                                w2t[:], w2[e, hs * P:(hs + 1) * P, os_])
                            w2ts.append(w2t)
                        for tt in range(NT):
                            ts_ = slice(tt * P, (tt + 1) * P)
                            py = ps2.tile([P, 512], dt.float32, tag="ps2")
                            for hs in range(NH):
                                nc.tensor.matmul(
                                    out=py[:], lhsT=hbuf[hs][:, ts_], rhs=w2ts[hs][:],
                                    start=(hs == 0), stop=(hs == NH - 1),
                                )
                            nc.vector.scalar_tensor_tensor(
                                out=outacc[tt][:, os_], in0=py[:],
                                scalar=coef[tt][:, e:e + 1], in1=outacc[tt][:, os_],
                                op0=ALU.mult, op1=ALU.add,
                            )

                for tt in range(NT):
                    nc.sync.dma_start(out[t0 + tt * P:t0 + (tt + 1) * P, :], outacc[tt][:])

    nc.compile()
    _NC_CACHE[key] = nc
    return nc


def prep_in_maps(x, Wr, br, W1, b1, W2, b2):
    x = np.ascontiguousarray(np.asarray(x, dtype=np.float32))
    Wr = np.ascontiguousarray(np.asarray(Wr, dtype=np.float32))
    br = np.asarray(br, dtype=np.float32)
    W1 = np.ascontiguousarray(np.asarray(W1, dtype=np.float32))
    b1 = np.asarray(b1, dtype=np.float32)
    W2 = np.ascontiguousarray(np.asarray(W2, dtype=np.float32))
    b2 = np.ascontiguousarray(np.asarray(b2, dtype=np.float32))
    xf = x.reshape(TOK, D)
    b1c = np.ascontiguousarray(b1.reshape(E, H // P, P).transpose(2, 0, 1).reshape(P, E * (H // P)))
    brc = np.ascontiguousarray(br.reshape(E, 1))
    MASK11 = np.uint32(0xFFFFF000)
    xhi = (xf.view(np.uint32) & MASK11).view(np.float32)
    xlo = xf - xhi
    wrhi = (Wr.view(np.uint32) & MASK11).view(np.float32)
    wrlo = Wr - wrhi
    in_maps = []
    for c in range(NCORES):
        sl = slice(c * T, (c + 1) * T)
        in_maps.append({
            "xT": np.ascontiguousarray(xf[sl].T),
            "xThi": np.ascontiguousarray(xhi[sl].T),
            "xTlo": np.ascontiguousarray(xlo[sl].T),
            "wrhi": wrhi, "wrlo": wrlo,
            "brc": brc, "w1": W1, "b1c": b1c, "w2": W2, "b2": b2,
        })
    return in_maps


def kernel(x, Wr, br, W1, b1, W2, b2):
    x = np.ascontiguousarray(np.asarray(x, dtype=np.float32))
    Wr = np.ascontiguousarray(np.asarray(Wr, dtype=np.float32))
    br = np.asarray(br, dtype=np.float32)
    W1 = np.ascontiguousarray(np.asarray(W1, dtype=np.float32))
    b1 = np.asarray(b1, dtype=np.float32)
    W2 = np.ascontiguousarray(np.asarray(W2, dtype=np.float32))
    b2 = np.ascontiguousarray(np.asarray(b2, dtype=np.float32))

    xf = x.reshape(TOK, D)
    b1c = np.ascontiguousarray(b1.reshape(E, H // P, P).transpose(2, 0, 1).reshape(P, E * (H // P)))
    brc = np.ascontiguousarray(br.reshape(E, 1))

    MASK11 = np.uint32(0xFFFFF000)
    xhi = (xf.view(np.uint32) & MASK11).view(np.float32)
    xlo = xf - xhi
    wrhi = (Wr.view(np.uint32) & MASK11).view(np.float32)
    wrlo = Wr - wrhi

    nc = build_nc()
    in_maps = []
    for c in range(NCORES):
        sl = slice(c * T, (c + 1) * T)
        in_maps.append({
            "xT": np.ascontiguousarray(xf[sl].T),
            "xThi": np.ascontiguousarray(xhi[sl].T),
            "xTlo": np.ascontiguousarray(xlo[sl].T),
            "wrhi": wrhi, "wrlo": wrlo,
            "brc": brc, "w1": W1, "b1c": b1c, "w2": W2, "b2": b2,
        })
    res = run_bass_kernel_spmd(nc, in_maps, core_ids=list(range(NCORES)))
    out = np.concatenate([res.results[c]["out"] for c in range(NCORES)], axis=0)
    return out.reshape(B, NOBJ, O)
